# revision 1
# baseline (speedup 1.0000x reference)
"""Trainium2 Bass kernel for nn_LRSVConv (low-rank spatially-varying conv).

Computes, for full inputs
    x            [8, 32, 256, 256]  f32
    conv_w       [192, 32, 3, 3]    f32   (192 = RANK(3) * C_OUT(64))
    kernel_weight[2, 256, 256]      f32
the reference:
    y   = conv2d(x, conv_w, stride 1, pad 1)      # [8, 192, 256, 256]
    y   = y.reshape(8, 3, 64, 256, 256)
    out = y[:,0] + kw[0]*y[:,1] + kw[1]*y[:,2]    # [8, 64, 256, 256]

Strategy: spatial (H) sharding across 8 cores - each core computes a band of
32 output rows for ALL batches, so the per-pixel blend weights (which are
batch-independent) are loaded/broadcast once per core and reused 8x.

Per core:
  - imcol tile [96, 32*258]: 3 kh-shifted replicas of the padded input rows
    (partition dim = (kh, c_in)), padded W=258 so kw shifts are free-dim
    offsets and no edge handling is needed.
  - conv: per supertile (4 output rows = 1024 px, split into 2 blocks of
    512 px), per rank r and kw: one K=96, M=64, N=512 fp32 matmul per block,
    the two blocks on opposite column halves of the PE array (concurrent via
    col tiling), accumulating in PSUM banks A/B/C (one per rank); psum rows
    = (block, c_out).
  - blend: t1 = B * sv1_bcast, t2 = C * sv2_bcast on DVE; t1 accumulated
    onto A via an identity matmul on the (otherwise busier) TensorE;
    out = A + t2 on DVE (fused PSUM evacuation).
  - sv broadcast tiles are prepared host-side ([128, 4096] per rank: rows
    (block, c) x band pixels) - tiny input, avoids on-device partition
    broadcast which no engine does well.
"""

import os

import numpy as np

B, C_IN, C_OUT, RANK, IMG = 8, 32, 64, 3, 256
N_CORES = 8
BAND = IMG // N_CORES          # 32 output rows per core
WP = IMG + 2                   # padded width 258
ROWS_IN = BAND + 2             # input rows needed per band (with halo)
SUPER = 8                      # supertiles per (batch, band): 4 rows each
SROWS = BAND // SUPER          # 4 image rows per supertile
NBLK = 512                     # pixels per matmul block (2 image rows)

_F32 = np.float32

# "pe": rank-1 partial added into PSUM A by an identity matmul on TensorE
# "dve": both adds on VectorE (simpler, more DVE load)
BLEND_MODE = os.environ.get("KERNEL_BLEND", "pe")
NB = int(os.environ.get("KERNEL_NB", str(B)))  # batches to process (debug knob)


def _build_bass():
    import concourse.mybir as mybir
    import concourse.tile as tile
    from concourse import bacc

    f32 = mybir.dt.float32
    # float32r: single-pass PE fp32 (1 cyc/row at N>=256 vs 4 for fp32)
    f32r = mybir.dt.float32r
    nc = bacc.Bacc("TRN2", target_bir_lowering=False, debug=False)

    xs_t = nc.dram_tensor("xs", (B, C_IN, ROWS_IN * WP), f32r, kind="ExternalInput")
    # wtBC[kw]: [96, (rank1|rank2)]; wtA[kw, q]: [96, (w0|0) or (0|w0)]
    wtbc_t = nc.dram_tensor("wtbc", (3, 96, 128), f32r, kind="ExternalInput")
    wta_t = nc.dram_tensor("wta", (3, 2, 96, 128), f32r, kind="ExternalInput")
    # S12: rows 0:64 = sv1, rows 64:128 = sv2; cols = (supertile, block, j)
    svb_t = nc.dram_tensor("svb", (128, SUPER * 2 * NBLK), f32, kind="ExternalInput")
    # identII[q]: cols 64q:64q+64 hold [I64; I64] (sum the two 64-row halves)
    id_t = nc.dram_tensor("ident", (2, 128, 128), f32r, kind="ExternalInput")
    out_t = nc.dram_tensor("out", (B, C_OUT, BAND, IMG), f32, kind="ExternalOutput")

    xs = xs_t.ap()
    out_r = out_t.ap().rearrange(
        "b c (t q r) w -> b q c t (r w)", t=SUPER, q=2, r=SROWS // 2
    )

    with tile.TileContext(nc) as tc:
        with (
            tc.tile_pool(name="const", bufs=1) as cpool,
            tc.tile_pool(name="imcol", bufs=2) as ipool,
            tc.tile_pool(name="psum", bufs=2, space="PSUM") as ppool,
            tc.tile_pool(name="tmp", bufs=3) as tpool,
            tc.tile_pool(name="outp", bufs=4) as opool,
        ):
            wtbc_sb = cpool.tile([96, 3, 128], f32r)
            nc.sync.dma_start(wtbc_sb[:], wtbc_t.ap().rearrange("k p m -> p k m"))
            wta_sb = cpool.tile([96, 3, 2, 128], f32r)
            nc.sync.dma_start(wta_sb[:], wta_t.ap().rearrange("k q p m -> p k q m"))
            svb_sb = cpool.tile([128, SUPER * 2 * NBLK], f32)
            nc.sync.dma_start(svb_sb[:], svb_t.ap())
            id_sb = cpool.tile([128, 2, 128], f32r)
            nc.sync.dma_start(id_sb[:], id_t.ap().rearrange("q p m -> p q m"))

            for b in range(NB):
                imcol = ipool.tile([96, BAND * WP], f32r, tag="imcol")
                for kh in range(3):
                    nc.sync.dma_start(
                        imcol[32 * kh : 32 * kh + 32, :],
                        xs[b, :, kh * WP : kh * WP + BAND * WP],
                    )
                imv = imcol.rearrange("p (h w) -> p h w", w=WP)

                for t in range(SUPER):
                    bc = ppool.tile([128, 2 * NBLK], f32, tag="bc")
                    a2 = ppool.tile([128, NBLK], f32, tag="a2")
                    for kw in range(3):
                        for q in range(2):
                            hl = SROWS * t + 2 * q
                            rhs = imv[:, hl : hl + 2, kw : kw + IMG]
                            nc.tensor.matmul(
                                bc[:, NBLK * q : NBLK * (q + 1)],
                                wtbc_sb[:, kw, :],
                                rhs,
                                start=(kw == 0),
                                stop=(kw == 2),
                            )
                            nc.tensor.matmul(
                                a2[:],
                                wta_sb[:, kw, q, :],
                                rhs,
                                start=(kw == 0 and q == 0),
                                stop=False,
                            )

                    # m = [sv1*y1 ; sv2*y2] for both blocks, one 128-row op
                    m = tpool.tile([128, 2 * NBLK], f32r, tag="m")
                    nc.vector.tensor_tensor(
                        m[:],
                        bc,
                        svb_sb[:, 2 * NBLK * t : 2 * NBLK * (t + 1)],
                        mybir.AluOpType.mult,
                    )
                    # fold the two 64-row halves of m into a2 rows (q*64..)
                    for q in range(2):
                        nc.tensor.matmul(
                            a2[:],
                            id_sb[:, q, :],
                            m[:, NBLK * q : NBLK * (q + 1)],
                            start=False,
                            stop=(q == 1),
                        )
                    out_sb = opool.tile([128, NBLK], f32, tag="out_sb")
                    nc.scalar.copy(out_sb[:], a2[:])
                    for q in range(2):
                        nc.sync.dma_start(
                            out_r[b, q, :, t, :], out_sb[64 * q : 64 * q + 64, :]
                        )
    nc.compile()
    return nc


_CACHE = {}


def _get_bass():
    if "nc" not in _CACHE:
        _CACHE["nc"] = _build_bass()
    return _CACHE["nc"]


def _prep_shards(x, conv_w, kernel_weight):
    x = np.asarray(x, dtype=_F32)
    conv_w = np.asarray(conv_w, dtype=_F32)
    kernel_weight = np.asarray(kernel_weight, dtype=_F32)

    x_pad = np.pad(x, ((0, 0), (0, 0), (1, 1), (1, 1)))
    # w[kh, c, kw, (r, m)] from conv_w[(r m), c, kh, kw]
    wt = conv_w.transpose(2, 1, 3, 0).reshape(96, 3, RANK * C_OUT)
    wtbc = np.ascontiguousarray(
        wt[:, :, C_OUT:].reshape(96, 3, 128).transpose(1, 0, 2)
    )  # [kw, 96, (r1|r2)]
    wta = np.zeros((3, 2, 96, 128), dtype=_F32)
    for q in range(2):
        wta[:, q, :, 64 * q : 64 * q + 64] = wt[:, :, :C_OUT].transpose(1, 0, 2)
    ident = np.zeros((2, 128, 128), dtype=_F32)
    for q in range(2):
        ident[q, 0:64, 64 * q : 64 * q + 64] = np.eye(64, dtype=_F32)
        ident[q, 64:128, 64 * q : 64 * q + 64] = np.eye(64, dtype=_F32)

    in_maps = []
    for i in range(N_CORES):
        h0 = BAND * i
        shard = np.ascontiguousarray(
            x_pad[:, :, h0 : h0 + ROWS_IN, :]
        ).reshape(B, C_IN, ROWS_IN * WP)
        band = kernel_weight[:, h0 : h0 + BAND, :]          # [2, 32, 256]
        # svb[64r+c, (t, q, j)] = band[r, row(t, q, j)]
        arr = band.reshape(2, SUPER, 2 * NBLK)              # [r, t, (q j)]
        svb = np.broadcast_to(
            arr[:, None, :, :], (2, C_OUT, SUPER, 2 * NBLK)
        ).reshape(128, SUPER * 2 * NBLK)
        svb = np.ascontiguousarray(svb)
        in_maps.append(
            {"xs": shard, "wtbc": wtbc, "wta": wta, "svb": svb, "ident": ident}
        )
    return in_maps


def run(inputs, trace=False):
    """Run the sharded bass kernel; returns (out_full, BassKernelResults)."""
    from concourse.bass_utils import run_bass_kernel_spmd

    in_maps = _prep_shards(**inputs)
    nc = _get_bass()
    res = run_bass_kernel_spmd(
        nc, in_maps, core_ids=list(range(N_CORES)), trace=trace
    )
    out = np.empty((B, C_OUT, IMG, IMG), dtype=_F32)
    for i in range(N_CORES):
        out[:, :, BAND * i : BAND * (i + 1), :] = res.results[i]["out"]
    return out, res


def kernel(x, conv_w, kernel_weight):
    out, _ = run({"x": x, "conv_w": conv_w, "kernel_weight": kernel_weight})
    return out



# revision 4
# speedup vs baseline: 1.7399x; 1.7399x over previous
"""Trainium2 Bass kernel for nn_LRSVConv (low-rank spatially-varying conv).

Computes, for full inputs
    x            [8, 32, 256, 256]  f32
    conv_w       [192, 32, 3, 3]    f32   (192 = RANK(3) * C_OUT(64))
    kernel_weight[2, 256, 256]      f32
the reference:
    y   = conv2d(x, conv_w, stride 1, pad 1)      # [8, 192, 256, 256]
    y   = y.reshape(8, 3, 64, 256, 256)
    out = y[:,0] + kw[0]*y[:,1] + kw[1]*y[:,2]    # [8, 64, 256, 256]

Sharding: spatial (H) across 8 cores - each core computes a band of 32
output rows for all batches.

Per-core kernel design (v2):
  - K=128 packing: partitions = (kh' in 0..3, c_in), where replica kh'
    holds the band's input rows shifted by kh' (even local rows only -
    odd output rows read odd input rows from the odd-kh' replicas).
    M=128 = (e, c_out) with e the output-row parity within a row pair:
    stationary W[(kh',ci),(e,c)] = conv_w[64r+c, ci, kh'-e, kw]
    (zero outside 0<=kh'-e<=2).  One matmul thus covers 2 output rows x
    64 channels at full 128-wide array occupancy.
  - Supertile = 4 output rows (2 row pairs p2) x 256 cols -> N=512.
    9 conv matmuls per supertile (3 ranks x 3 kw): ranks 1,2 into one
    2-bank psum bc2 [128,(rk,p2,j)]; rank 0 into psum A [128,512].
  - Blend is partition-aligned (rows are (e,c) for every rank):
      DVE:    m  = bc2 * sv12      (per-pixel weights, host-broadcast)
      GpSimd: s  = m[:,:512] + m[:,512:]
      PE:     A += I @ s           (identity matmul, N=512)
      Act:    osb = copy(A); DMA out.
    The identity matmul of supertile t is emitted after the conv
    matmuls of supertile t+1 (software pipelining) so the PE never
    waits on the DVE/GpSimd chain - keeping the HAM clock gate at
    2.4 GHz instead of 1.2.
  - imcol + weights in bf16 (psum accumulation stays f32): halves DMA
    traffic and enables fast weight load; well within the accuracy
    budget.
"""

import os

import ml_dtypes
import numpy as np

B, C_IN, C_OUT, RANK, IMG = 8, 32, 64, 3, 256
N_CORES = 8
BAND = IMG // N_CORES          # 32 output rows per core
WP = IMG + 2                   # padded width 258
NL = 16                        # even-local-row slots per partition
T = 8                          # supertiles per (batch, band): 4 rows each
NBLK = 512                     # matmul free size: (p2=2) x (j=256)

_F32 = np.float32
_BF16 = ml_dtypes.bfloat16

# "pe": fold s into rank-0 psum with an identity matmul, Act evacuates.
# "dve": out = A + s directly on DVE (no identity matmul, no Act).
BLEND = os.environ.get("KERNEL_BLEND", "pe")


def _build_bass():
    import concourse.mybir as mybir
    import concourse.tile as tile
    from concourse import bacc

    f32 = mybir.dt.float32
    bf16 = mybir.dt.bfloat16
    nc = bacc.Bacc("TRN2", target_bir_lowering=False, debug=False)

    xs_t = nc.dram_tensor("xs", (B, 128, NL * WP), bf16, kind="ExternalInput")
    w_t = nc.dram_tensor("wconv", (RANK, 3, 128, 128), bf16, kind="ExternalInput")
    id_t = nc.dram_tensor("ident", (128, 128), bf16, kind="ExternalInput")
    sv_t = nc.dram_tensor("sv", (128, T * 2 * NBLK), f32, kind="ExternalInput")
    out_t = nc.dram_tensor("out", (B, C_OUT, BAND, IMG), f32, kind="ExternalOutput")

    xs = xs_t.ap()
    # band row = 4t + 2*p2 + e ; psum/sbuf rows are (e,c), cols (p2,j)
    out_r = out_t.ap().rearrange(
        "b c (t p2 e) w -> b e c t p2 w", t=T, p2=2, e=2
    )

    with tile.TileContext(nc) as tc:
        with (
            tc.tile_pool(name="const", bufs=1) as cpool,
            tc.tile_pool(name="imcol", bufs=2) as ipool,
            tc.tile_pool(name="psum", bufs=2, space="PSUM") as ppool,
            tc.tile_pool(name="mb", bufs=3) as mpool,
            tc.tile_pool(name="sb", bufs=3) as spool,
            tc.tile_pool(name="outp", bufs=4) as opool,
        ):
            w_sb = cpool.tile([128, RANK, 3, 128], bf16)
            nc.sync.dma_start(w_sb[:], w_t.ap().rearrange("r k p m -> p r k m"))
            id_sb = cpool.tile([128, 128], bf16)
            nc.sync.dma_start(id_sb[:], id_t.ap())
            sv_sb = cpool.tile([128, T * 2 * NBLK], f32)
            nc.sync.dma_start(sv_sb[:], sv_t.ap())

            def retire(pending):
                ap_, s, pb, pt = pending
                osb = opool.tile([128, NBLK], f32, tag="osb")
                if BLEND == "pe":
                    nc.tensor.matmul(ap_[:], id_sb[:], s[:], start=False, stop=True)
                    nc.scalar.copy(osb[:], ap_[:])
                else:
                    nc.vector.tensor_tensor(
                        osb[:], ap_[:], s[:], mybir.AluOpType.add
                    )
                for e in range(2):
                    nc.sync.dma_start(
                        out_r[pb, e, :, pt], osb[64 * e : 64 * e + 64, :]
                    )

            pending = None
            for b in range(B):
                im = ipool.tile([128, NL * WP], bf16, tag="im")
                nc.sync.dma_start(im[:], xs[b])
                imv = im.rearrange("p (l w) -> p l w", w=WP)
                for t in range(T):
                    bc2 = ppool.tile([128, 2 * NBLK], f32, tag="bc2")
                    ap_ = ppool.tile([128, NBLK], f32, tag="acc")
                    for r in (1, 2):
                        for kw in range(3):
                            nc.tensor.matmul(
                                bc2[:, NBLK * (r - 1) : NBLK * r],
                                w_sb[:, r, kw, :],
                                imv[:, 2 * t : 2 * t + 2, kw : kw + IMG],
                                start=(kw == 0),
                                stop=(kw == 2),
                            )
                    for kw in range(3):
                        nc.tensor.matmul(
                            ap_[:],
                            w_sb[:, 0, kw, :],
                            imv[:, 2 * t : 2 * t + 2, kw : kw + IMG],
                            start=(kw == 0),
                            stop=(BLEND != "pe" and kw == 2),
                        )
                    m = mpool.tile([128, 2 * NBLK], bf16, tag="m")
                    nc.vector.tensor_tensor(
                        m[:],
                        bc2[:],
                        sv_sb[:, 2 * NBLK * t : 2 * NBLK * (t + 1)],
                        mybir.AluOpType.mult,
                    )
                    s = spool.tile([128, NBLK], bf16, tag="s")
                    nc.gpsimd.tensor_add(s[:], m[:, 0:NBLK], m[:, NBLK : 2 * NBLK])
                    if pending is not None:
                        retire(pending)
                    pending = (ap_, s, b, t)
            retire(pending)
    nc.compile()
    return nc


_CACHE = {}


def _get_bass():
    if "nc" not in _CACHE:
        _CACHE["nc"] = _build_bass()
    return _CACHE["nc"]


def _prep_shards(x, conv_w, kernel_weight):
    x = np.asarray(x, dtype=_F32)
    conv_w = np.asarray(conv_w, dtype=_F32)
    kernel_weight = np.asarray(kernel_weight, dtype=_F32)

    x_pad = np.pad(x, ((0, 0), (0, 0), (1, 1), (1, 1)))  # [B,32,258,258]

    # stationary: w[r, kw, (kh',ci), (e,c)] = conv_w[64r+c, ci, kh'-e, kw]
    cw5 = conv_w.reshape(RANK, C_OUT, C_IN, 3, 3)
    w = np.zeros((RANK, 3, 4, C_IN, 2, C_OUT), dtype=_F32)
    for e in range(2):
        for khp in range(4):
            kh = khp - e
            if 0 <= kh <= 2:
                # cw5[:, c, ci, kh, kw] -> (r, kw, ci, c)
                w[:, :, khp, :, e, :] = cw5[:, :, :, kh, :].transpose(0, 3, 2, 1)
    wfull = w.reshape(RANK, 3, 128, 128).astype(_BF16)

    ident = np.eye(128, dtype=_F32).astype(_BF16)

    in_maps = []
    for i in range(N_CORES):
        h0 = BAND * i
        # xs[b, 32*khp+ci, lr2, w] = x_pad[b, ci, h0 + 2*lr2 + khp, w]
        xband = x_pad[:, :, h0 : h0 + BAND + 2, :]  # [B,32,34,258]
        xsh = np.empty((B, 4, C_IN, NL, WP), dtype=_BF16)
        for khp in range(4):
            xsh[:, khp] = xband[:, :, khp : khp + 32 : 2, :]
        xsh = xsh.reshape(B, 128, NL * WP)

        # sv[(e,c), (t,rk,p2,j)] = kernel_weight[rk, h0+4t+2p2+e, j]
        kb = kernel_weight[:, h0 : h0 + BAND, :].reshape(2, T, 2, 2, IMG)
        svb = kb.transpose(3, 1, 0, 2, 4)  # [e, t, rk, p2, j]
        svb = np.broadcast_to(
            svb[:, None], (2, C_OUT, T, 2, 2, IMG)
        ).reshape(128, T * 2 * NBLK)
        svb = np.ascontiguousarray(svb)

        in_maps.append(
            {"xs": xsh, "wconv": wfull, "ident": ident, "sv": svb}
        )
    return in_maps


def run(inputs, trace=False):
    """Run the sharded bass kernel; returns (out_full, BassKernelResults)."""
    from concourse.bass_utils import run_bass_kernel_spmd

    in_maps = _prep_shards(**inputs)
    nc = _get_bass()
    res = run_bass_kernel_spmd(
        nc, in_maps, core_ids=list(range(N_CORES)), trace=trace
    )
    out = np.empty((B, C_OUT, IMG, IMG), dtype=_F32)
    for i in range(N_CORES):
        out[:, :, BAND * i : BAND * (i + 1), :] = res.results[i]["out"]
    return out, res


def kernel(x, conv_w, kernel_weight):
    out, _ = run({"x": x, "conv_w": conv_w, "kernel_weight": kernel_weight})
    return out


# revision 14
# speedup vs baseline: 2.1257x; 1.2217x over previous
"""Trainium2 Bass kernel for nn_LRSVConv (low-rank spatially-varying conv).

Computes, for full inputs
    x            [8, 32, 256, 256]  f32
    conv_w       [192, 32, 3, 3]    f32   (192 = RANK(3) * C_OUT(64))
    kernel_weight[2, 256, 256]      f32
the reference:
    y   = conv2d(x, conv_w, stride 1, pad 1)      # [8, 192, 256, 256]
    y   = y.reshape(8, 3, 64, 256, 256)
    out = y[:,0] + kw[0]*y[:,1] + kw[1]*y[:,2]    # [8, 64, 256, 256]

Sharding: spatial (H) across 8 cores - each core computes a band of 32
output rows for all batches.

Per-core kernel design (v2):
  - K=128 packing: partitions = (kh' in 0..3, c_in), where replica kh'
    holds the band's input rows shifted by kh' (even local rows only -
    odd output rows read odd input rows from the odd-kh' replicas).
    M=128 = (e, c_out) with e the output-row parity within a row pair:
    stationary W[(kh',ci),(e,c)] = conv_w[64r+c, ci, kh'-e, kw]
    (zero outside 0<=kh'-e<=2).  One matmul thus covers 2 output rows x
    64 channels at full 128-wide array occupancy.
  - Supertile = 4 output rows (2 row pairs p2) x 256 cols -> N=512.
    9 conv matmuls per supertile (3 ranks x 3 kw): ranks 1,2 into one
    2-bank psum bc2 [128,(rk,p2,j)]; rank 0 into psum A [128,512].
  - Blend is partition-aligned (rows are (e,c) for every rank):
      DVE:    m  = bc2 * sv12      (per-pixel weights, host-broadcast)
      GpSimd: s  = m[:,:512] + m[:,512:]
      PE:     A += I @ s           (identity matmul, N=512)
      Act:    osb = copy(A); DMA out.
    The identity matmul of supertile t is emitted after the conv
    matmuls of supertile t+1 (software pipelining) so the PE never
    waits on the DVE/GpSimd chain - keeping the HAM clock gate at
    2.4 GHz instead of 1.2.
  - imcol + weights in bf16 (psum accumulation stays f32): halves DMA
    traffic and enables fast weight load; well within the accuracy
    budget.
"""

import os

import ml_dtypes
import numpy as np

B, C_IN, C_OUT, RANK, IMG = 8, 32, 64, 3, 256
N_CORES = 8
BAND = IMG // N_CORES          # 32 output rows per core
WP = IMG + 2                   # padded width 258
NL = 16                        # even-local-row slots per partition
T = 8                          # supertiles per (batch, band): 4 rows each
NBLK = 512                     # matmul free size: (p2=2) x (j=256)

_F32 = np.float32
_BF16 = ml_dtypes.bfloat16

# "pe": fold s into rank-0 psum with an identity matmul, Act evacuates.
# "dve": out = A + s directly on DVE (no identity matmul, no Act).
BLEND = os.environ.get("KERNEL_BLEND", "pe")


def _build_bass():
    import concourse.mybir as mybir
    import concourse.tile as tile
    from concourse import bacc

    f32 = mybir.dt.float32
    bf16 = mybir.dt.bfloat16
    nc = bacc.Bacc("TRN2", target_bir_lowering=False, debug=False)

    xs_t = nc.dram_tensor("xs", (B, 128, NL * WP), bf16, kind="ExternalInput")
    w_t = nc.dram_tensor("wconv", (RANK, 3, 128, 128), bf16, kind="ExternalInput")
    id_t = nc.dram_tensor("ident", (128, 128), bf16, kind="ExternalInput")
    sv_t = nc.dram_tensor("sv", (128, T * 2 * NBLK), bf16, kind="ExternalInput")
    out_t = nc.dram_tensor("out", (B, C_OUT, BAND, IMG), f32, kind="ExternalOutput")

    xs = xs_t.ap()
    # band row = 4t + 2*p2 + e ; psum/sbuf rows are (e,c), cols (p2,j)
    out_r = out_t.ap().rearrange(
        "b c (t p2 e) w -> b e c t p2 w", t=T, p2=2, e=2
    )

    with tile.TileContext(nc) as tc:
        with (
            tc.tile_pool(name="const", bufs=1) as cpool,
            tc.tile_pool(name="imcol", bufs=2) as ipool,
            tc.tile_pool(name="psum", bufs=2, space="PSUM") as ppool,
            tc.tile_pool(name="mb", bufs=3) as mpool,
            tc.tile_pool(name="sb", bufs=3) as spool,
            tc.tile_pool(name="outp", bufs=4) as opool,
        ):
            w_sb = cpool.tile([128, RANK, 3, 128], bf16)
            nc.sync.dma_start(w_sb[:], w_t.ap().rearrange("r k p m -> p r k m"))
            id_sb = cpool.tile([128, 128], bf16)
            nc.sync.dma_start(id_sb[:], id_t.ap())
            sv_sb = cpool.tile([128, T * 2 * NBLK], bf16)
            nc.sync.dma_start(sv_sb[:], sv_t.ap())

            def retire(pending):
                ap_, s, pb, pt = pending
                osb = opool.tile([128, NBLK], f32, tag="osb")
                if BLEND == "pe":
                    nc.tensor.matmul(ap_[:], id_sb[:], s[:], start=False, stop=True)
                    nc.scalar.copy(osb[:], ap_[:])
                else:
                    nc.vector.tensor_tensor(
                        osb[:], ap_[:], s[:], mybir.AluOpType.add
                    )
                for e in range(2):
                    nc.sync.dma_start(
                        out_r[pb, e, :, pt], osb[64 * e : 64 * e + 64, :]
                    )

            pending = None
            for b in range(B):
                im = ipool.tile([128, NL * WP], bf16, tag="im")
                nc.sync.dma_start(im[:], xs[b])
                imv = im.rearrange("p (l w) -> p l w", w=WP)
                for t in range(T):
                    bc2 = ppool.tile([128, 2 * NBLK], f32, tag="bc2")
                    ap_ = ppool.tile([128, NBLK], f32, tag="acc")
                    for r in (1, 2):
                        for kw in range(3):
                            nc.tensor.matmul(
                                bc2[:, NBLK * (r - 1) : NBLK * r],
                                w_sb[:, r, kw, :],
                                imv[:, 2 * t : 2 * t + 2, kw : kw + IMG],
                                start=(kw == 0),
                                stop=(kw == 2),
                            )
                    for kw in range(3):
                        nc.tensor.matmul(
                            ap_[:],
                            w_sb[:, 0, kw, :],
                            imv[:, 2 * t : 2 * t + 2, kw : kw + IMG],
                            start=(kw == 0),
                            stop=(BLEND != "pe" and kw == 2),
                        )
                    m = mpool.tile([128, 2 * NBLK], bf16, tag="m")
                    nc.vector.tensor_tensor(
                        m[:],
                        bc2[:],
                        sv_sb[:, 2 * NBLK * t : 2 * NBLK * (t + 1)],
                        mybir.AluOpType.mult,
                    )
                    s = spool.tile([128, NBLK], bf16, tag="s")
                    nc.gpsimd.tensor_add(s[:], m[:, 0:NBLK], m[:, NBLK : 2 * NBLK])
                    if pending is not None:
                        retire(pending)
                    pending = (ap_, s, b, t)
            retire(pending)
    nc.compile()
    return nc


_CACHE = {}


def _get_bass():
    if "nc" not in _CACHE:
        _CACHE["nc"] = _build_bass()
    return _CACHE["nc"]


def _prep_shards(x, conv_w, kernel_weight):
    x = np.asarray(x, dtype=_F32)
    conv_w = np.asarray(conv_w, dtype=_F32)
    kernel_weight = np.asarray(kernel_weight, dtype=_F32)

    x_pad = np.pad(x, ((0, 0), (0, 0), (1, 1), (1, 1)))  # [B,32,258,258]

    # stationary: w[r, kw, (kh',ci), (e,c)] = conv_w[64r+c, ci, kh'-e, kw]
    cw5 = conv_w.reshape(RANK, C_OUT, C_IN, 3, 3)
    w = np.zeros((RANK, 3, 4, C_IN, 2, C_OUT), dtype=_F32)
    for e in range(2):
        for khp in range(4):
            kh = khp - e
            if 0 <= kh <= 2:
                # cw5[:, c, ci, kh, kw] -> (r, kw, ci, c)
                w[:, :, khp, :, e, :] = cw5[:, :, :, kh, :].transpose(0, 3, 2, 1)
    wfull = w.reshape(RANK, 3, 128, 128).astype(_BF16)

    ident = np.eye(128, dtype=_F32).astype(_BF16)

    in_maps = []
    for i in range(N_CORES):
        h0 = BAND * i
        # xs[b, 32*khp+ci, lr2, w] = x_pad[b, ci, h0 + 2*lr2 + khp, w]
        xband = x_pad[:, :, h0 : h0 + BAND + 2, :]  # [B,32,34,258]
        xsh = np.empty((B, 4, C_IN, NL, WP), dtype=_BF16)
        for khp in range(4):
            xsh[:, khp] = xband[:, :, khp : khp + 32 : 2, :]
        xsh = xsh.reshape(B, 128, NL * WP)

        # sv[(e,c), (t,rk,p2,j)] = kernel_weight[rk, h0+4t+2p2+e, j]
        kb = kernel_weight[:, h0 : h0 + BAND, :].reshape(2, T, 2, 2, IMG)
        svb = kb.transpose(3, 1, 0, 2, 4)  # [e, t, rk, p2, j]
        svb = np.broadcast_to(
            svb[:, None], (2, C_OUT, T, 2, 2, IMG)
        ).reshape(128, T * 2 * NBLK)
        svb = np.ascontiguousarray(svb).astype(_BF16)

        in_maps.append(
            {"xs": xsh, "wconv": wfull, "ident": ident, "sv": svb}
        )
    return in_maps


def run(inputs, trace=False):
    """Run the sharded bass kernel; returns (out_full, BassKernelResults)."""
    from concourse.bass_utils import run_bass_kernel_spmd

    in_maps = _prep_shards(**inputs)
    nc = _get_bass()
    res = run_bass_kernel_spmd(
        nc, in_maps, core_ids=list(range(N_CORES)), trace=trace
    )
    out = np.empty((B, C_OUT, IMG, IMG), dtype=_F32)
    for i in range(N_CORES):
        out[:, :, BAND * i : BAND * (i + 1), :] = res.results[i]["out"]
    return out, res


def kernel(x, conv_w, kernel_weight):
    out, _ = run({"x": x, "conv_w": conv_w, "kernel_weight": kernel_weight})
    return out


# revision 20
# speedup vs baseline: 2.2153x; 1.0422x over previous
"""Trainium2 Bass kernel for nn_LRSVConv (low-rank spatially-varying conv).

Computes, for full inputs
    x            [8, 32, 256, 256]  f32
    conv_w       [192, 32, 3, 3]    f32   (192 = RANK(3) * C_OUT(64))
    kernel_weight[2, 256, 256]      f32
the reference:
    y   = conv2d(x, conv_w, stride 1, pad 1)      # [8, 192, 256, 256]
    y   = y.reshape(8, 3, 64, 256, 256)
    out = y[:,0] + kw[0]*y[:,1] + kw[1]*y[:,2]    # [8, 64, 256, 256]

Sharding: spatial (H) across 8 cores - each core computes a band of 32
output rows for all batches.

Per-core kernel design (v2):
  - K=128 packing: partitions = (kh' in 0..3, c_in), where replica kh'
    holds the band's input rows shifted by kh' (even local rows only -
    odd output rows read odd input rows from the odd-kh' replicas).
    M=128 = (e, c_out) with e the output-row parity within a row pair:
    stationary W[(kh',ci),(e,c)] = conv_w[64r+c, ci, kh'-e, kw]
    (zero outside 0<=kh'-e<=2).  One matmul thus covers 2 output rows x
    64 channels at full 128-wide array occupancy.
  - Supertile = 4 output rows (2 row pairs p2) x 256 cols -> N=512.
    9 conv matmuls per supertile (3 ranks x 3 kw): ranks 1,2 into one
    2-bank psum bc2 [128,(rk,p2,j)]; rank 0 into psum A [128,512].
  - Blend is partition-aligned (rows are (e,c) for every rank):
      DVE:    m  = bc2 * sv12      (per-pixel weights, host-broadcast)
      GpSimd: s  = m[:,:512] + m[:,512:]
      PE:     A += I @ s           (identity matmul, N=512)
      Act:    osb = copy(A); DMA out.
    The identity matmul of supertile t is emitted after the conv
    matmuls of supertile t+1 (software pipelining) so the PE never
    waits on the DVE/GpSimd chain - keeping the HAM clock gate at
    2.4 GHz instead of 1.2.
  - imcol + weights in bf16 (psum accumulation stays f32): halves DMA
    traffic and enables fast weight load; well within the accuracy
    budget.
"""

import os

import ml_dtypes
import numpy as np

B, C_IN, C_OUT, RANK, IMG = 8, 32, 64, 3, 256
N_CORES = 8
BAND = IMG // N_CORES          # 32 output rows per core
WP = IMG + 2                   # padded width 258
NL = 16                        # even-local-row slots per partition
T = 8                          # supertiles per (batch, band): 4 rows each
NBLK = 512                     # matmul free size: (p2=2) x (j=256)

_F32 = np.float32
_BF16 = ml_dtypes.bfloat16

# "act": Act evacuates rank-0 psum to bf16 SBUF, DVE adds (bf16 2x mode).
# "pe":  fold s into rank-0 psum with an identity matmul, Act evacuates.
# "dve": out = A + s directly on DVE (psum operand, 1x mode).
BLEND = os.environ.get("KERNEL_BLEND", "act")
RETIRE_DIST = int(os.environ.get("KERNEL_RETIRE_DIST", "2"))


def _build_bass():
    import concourse.mybir as mybir
    import concourse.tile as tile
    from concourse import bacc

    f32 = mybir.dt.float32
    bf16 = mybir.dt.bfloat16
    nc = bacc.Bacc("TRN2", target_bir_lowering=False, debug=False)

    xs_t = nc.dram_tensor("xs", (B, 128, NL * WP), bf16, kind="ExternalInput")
    w_t = nc.dram_tensor("wconv", (RANK, 3, 128, 128), bf16, kind="ExternalInput")
    id_t = nc.dram_tensor("ident", (128, 128), bf16, kind="ExternalInput")
    sv_t = nc.dram_tensor("sv", (128, T * 2 * NBLK), bf16, kind="ExternalInput")
    out_t = nc.dram_tensor("out", (B, C_OUT, BAND, IMG), bf16, kind="ExternalOutput")

    xs = xs_t.ap()
    # band row = 4t + 2*p2 + e ; psum/sbuf rows are (e,c), cols (p2,j)
    out_r = out_t.ap().rearrange(
        "b c (t p2 e) w -> b e c t p2 w", t=T, p2=2, e=2
    )

    with tile.TileContext(nc) as tc:
        with (
            tc.tile_pool(name="const", bufs=1) as cpool,
            tc.tile_pool(name="imcol", bufs=2) as ipool,
            tc.tile_pool(name="psum", bufs=2, space="PSUM") as ppool,
            tc.tile_pool(name="psacc", bufs=RETIRE_DIST + 1, space="PSUM") as papool,
            tc.tile_pool(name="mb", bufs=3) as mpool,
            tc.tile_pool(name="sb", bufs=RETIRE_DIST + 2) as spool,
            tc.tile_pool(name="ab", bufs=3) as apool,
            tc.tile_pool(name="outp", bufs=4) as opool,
        ):
            w_sb = cpool.tile([128, RANK, 3, 128], bf16)
            nc.sync.dma_start(w_sb[:], w_t.ap().rearrange("r k p m -> p r k m"))
            id_sb = cpool.tile([128, 128], bf16)
            nc.sync.dma_start(id_sb[:], id_t.ap())
            sv_sb = cpool.tile([128, T * 2 * NBLK], bf16)
            nc.sync.dma_start(sv_sb[:], sv_t.ap())

            def retire(pending):
                ap_, s, pb, pt = pending
                osb = opool.tile([128, NBLK], bf16, tag="osb")
                if BLEND == "pe":
                    nc.tensor.matmul(ap_[:], id_sb[:], s[:], start=False, stop=True)
                    nc.scalar.copy(osb[:], ap_[:])
                elif BLEND == "act":
                    a_bf = apool.tile([128, NBLK], bf16, tag="a_bf")
                    nc.scalar.copy(a_bf[:], ap_[:])
                    nc.vector.tensor_tensor(
                        osb[:], a_bf[:], s[:], mybir.AluOpType.add
                    )
                else:
                    nc.vector.tensor_tensor(
                        osb[:], ap_[:], s[:], mybir.AluOpType.add
                    )
                for e in range(2):
                    nc.sync.dma_start(
                        out_r[pb, e, :, pt], osb[64 * e : 64 * e + 64, :]
                    )

            pending = []
            for b in range(B):
                im = ipool.tile([128, NL * WP], bf16, tag="im")
                nc.sync.dma_start(im[:], xs[b])
                imv = im.rearrange("p (l w) -> p l w", w=WP)
                for t in range(T):
                    bc2 = ppool.tile([128, 2 * NBLK], f32, tag="bc2")
                    ap_ = papool.tile([128, NBLK], f32, tag="acc")
                    for r in (1, 2):
                        for kw in range(3):
                            nc.tensor.matmul(
                                bc2[:, NBLK * (r - 1) : NBLK * r],
                                w_sb[:, r, kw, :],
                                imv[:, 2 * t : 2 * t + 2, kw : kw + IMG],
                                start=(kw == 0),
                                stop=(kw == 2),
                            )
                    for kw in range(3):
                        nc.tensor.matmul(
                            ap_[:],
                            w_sb[:, 0, kw, :],
                            imv[:, 2 * t : 2 * t + 2, kw : kw + IMG],
                            start=(kw == 0),
                            stop=(BLEND != "pe" and kw == 2),
                        )
                    m = mpool.tile([128, 2 * NBLK], bf16, tag="m")
                    nc.vector.tensor_tensor(
                        m[:],
                        bc2[:],
                        sv_sb[:, 2 * NBLK * t : 2 * NBLK * (t + 1)],
                        mybir.AluOpType.mult,
                    )
                    s = spool.tile([128, NBLK], bf16, tag="s")
                    nc.gpsimd.tensor_add(s[:], m[:, 0:NBLK], m[:, NBLK : 2 * NBLK])
                    pending.append((ap_, s, b, t))
                    if len(pending) > RETIRE_DIST:
                        retire(pending.pop(0))
            for p in pending:
                retire(p)
    nc.compile()
    return nc


_CACHE = {}


def _get_bass():
    if "nc" not in _CACHE:
        _CACHE["nc"] = _build_bass()
    return _CACHE["nc"]


def _prep_shards(x, conv_w, kernel_weight):
    x = np.asarray(x, dtype=_F32)
    conv_w = np.asarray(conv_w, dtype=_F32)
    kernel_weight = np.asarray(kernel_weight, dtype=_F32)

    x_pad = np.pad(x, ((0, 0), (0, 0), (1, 1), (1, 1)))  # [B,32,258,258]

    # stationary: w[r, kw, (kh',ci), (e,c)] = conv_w[64r+c, ci, kh'-e, kw]
    cw5 = conv_w.reshape(RANK, C_OUT, C_IN, 3, 3)
    w = np.zeros((RANK, 3, 4, C_IN, 2, C_OUT), dtype=_F32)
    for e in range(2):
        for khp in range(4):
            kh = khp - e
            if 0 <= kh <= 2:
                # cw5[:, c, ci, kh, kw] -> (r, kw, ci, c)
                w[:, :, khp, :, e, :] = cw5[:, :, :, kh, :].transpose(0, 3, 2, 1)
    wfull = w.reshape(RANK, 3, 128, 128).astype(_BF16)

    ident = np.eye(128, dtype=_F32).astype(_BF16)

    in_maps = []
    for i in range(N_CORES):
        h0 = BAND * i
        # xs[b, 32*khp+ci, lr2, w] = x_pad[b, ci, h0 + 2*lr2 + khp, w]
        xband = x_pad[:, :, h0 : h0 + BAND + 2, :]  # [B,32,34,258]
        xsh = np.empty((B, 4, C_IN, NL, WP), dtype=_BF16)
        for khp in range(4):
            xsh[:, khp] = xband[:, :, khp : khp + 32 : 2, :]
        xsh = xsh.reshape(B, 128, NL * WP)

        # sv[(e,c), (t,rk,p2,j)] = kernel_weight[rk, h0+4t+2p2+e, j]
        kb = kernel_weight[:, h0 : h0 + BAND, :].reshape(2, T, 2, 2, IMG)
        svb = kb.transpose(3, 1, 0, 2, 4)  # [e, t, rk, p2, j]
        svb = np.broadcast_to(
            svb[:, None], (2, C_OUT, T, 2, 2, IMG)
        ).reshape(128, T * 2 * NBLK)
        svb = np.ascontiguousarray(svb).astype(_BF16)

        in_maps.append(
            {"xs": xsh, "wconv": wfull, "ident": ident, "sv": svb}
        )
    return in_maps


def run(inputs, trace=False):
    """Run the sharded bass kernel; returns (out_full, BassKernelResults)."""
    from concourse.bass_utils import run_bass_kernel_spmd

    in_maps = _prep_shards(**inputs)
    nc = _get_bass()
    res = run_bass_kernel_spmd(
        nc, in_maps, core_ids=list(range(N_CORES)), trace=trace
    )
    out = np.empty((B, C_OUT, IMG, IMG), dtype=_F32)
    for i in range(N_CORES):
        out[:, :, BAND * i : BAND * (i + 1), :] = res.results[i]["out"].astype(_F32)
    return out, res


def kernel(x, conv_w, kernel_weight):
    out, _ = run({"x": x, "conv_w": conv_w, "kernel_weight": kernel_weight})
    return out
